# revision 3
# baseline (speedup 1.0000x reference)
"""3-layer GCN (PyG GCNConv semantics) on 8 Trainium2 NeuronCores — v5 (projected layer-3 exchange).

vs v2: source-row gathers use the custom InstDMAGatherAnt
(gpsimd.dma_gather, mlp library) instead of generic indirect DMAs.
Generic indirect DMA costs ~1.2us of SWDGE launch per 128-row call
(2352 calls ~ 2.8ms); dma_gather batches ~900 rows per call.

Design, shaped by dma_gather's constraints (int16 indices => <=32k-row
source windows; lanes pack partition-major in 128-lane chunks):
  - Sources split into 7 address WINDOWS of exactly 28672 rows.
  - Edge lanes (self-loops EXCLUDED) are grouped into cells
    (dest tile-PAIR, window), each cell padded to 128 lanes (pad lanes
    idx 0 / nrm 0). Cell chunk counts are made identical across cores
    (max; in practice 1 chunk/cell) so the single SPMD program fits
    every core.
  - Self-loop messages need the core's OWN rows only: per pair, one
    strided HWDGE dma_start pulls its 256 own-shard rows into 2
    dedicated chunks — no indexed gather at all.
  - One dma_gather call per (group of 7 pairs, window) => 98 calls +
    14 self DMAs per layer into a double-buffered staging tile.
  - Scatter: per pair, per 128-row feature slice, pa[fw, 256] (PSUM)
    accumulates over the pair's chunks: lhsT = staged lanes (bf16,
    fast weight load), rhs = one-hot mh [128, 256] built by one
    VectorE tensor_scalar per chunk. ScalarE casts pa to bf16 aggT.
  - Transform (row-major, no transposes): out[dest, j] accumulates
    lhsT = aggT f-slices, rhs = W row-blocks (N=384); VectorE adds the
    bias in PSUM; ScalarE fuses ReLU into the PSUM->SBUF bf16 cast.
  - One AllGather per layer exchanges activation shards (Shared-space
    DRAM, single collective — chunked AG is rejected by the
    single-writer rule on Shared tensors).

kernel(**inputs) takes FULL unsharded inputs, returns FULL [200000, 2]
float32 output.
"""

import time
from contextlib import ExitStack

import numpy as np

import concourse.bass as bass
import concourse.mybir as mybir
import concourse.tile as tile
from concourse import bacc
from concourse import bass_utils
from concourse import library_config
from concourse._compat import axon_active

P = 128
F32 = mybir.dt.float32
BF16 = mybir.dt.bfloat16
I16 = mybir.dt.int16

N_NODES = 200000
F_IN = 165
F_IN_PAD = 256        # x rows padded (dma_gather needs elem bytes %256)
HIDDEN = 384
F_OUT = 2
N_CORES = 8
TILES_PER_CORE = 196
PAIRS = TILES_PER_CORE // 2          # 98
GROUP_PAIRS = 7
N_GROUPS = PAIRS // GROUP_PAIRS      # 14
SEGR = TILES_PER_CORE * P            # core shard rows = 25088
N_WIN = 7
WINR = 28672                         # 7 * 28672 = 200704 = n_pad
ZPAD = 128                           # z rows padded to 128 bf16 = 256B

LAST_RESULTS = None
EXEC_NS = None

import os as _os
NO_AG = int(_os.environ.get("KV3_NOAG", "0"))  # timing experiment only


def _ceil_div(a, b):
    return (a + b - 1) // b


# --------------------------------------------------------------------------
# host-side preprocessing
# --------------------------------------------------------------------------

def _preprocess(edge_index, n_nodes):
    n_bins = N_CORES * TILES_PER_CORE
    n_pad = n_bins * P
    assert n_pad == N_WIN * WINR

    row = np.asarray(edge_index[0], dtype=np.int64)
    col = np.asarray(edge_index[1], dtype=np.int64)
    loops = np.arange(n_nodes, dtype=np.int64)
    col_deg = np.concatenate([col, loops])
    deg = np.bincount(col_deg, minlength=n_nodes).astype(np.float64)  # >= 1
    dinv = 1.0 / np.sqrt(deg)

    # node -> (bin, slot): serpentine over bins in descending-degree order
    d = np.zeros(n_pad, np.int64)
    d[:n_nodes] = deg.astype(np.int64)
    order = np.argsort(-d, kind="stable")
    rows_idx = np.arange(n_pad) // n_bins
    pos = np.arange(n_pad) % n_bins
    bins_of_rank = np.where(rows_idx % 2 == 0, pos, n_bins - 1 - pos)
    bin_of_node = np.empty(n_pad, np.int64)
    slot_of_node = np.empty(n_pad, np.int64)
    bin_of_node[order] = bins_of_rank
    slot_of_node[order] = rows_idx
    perm = bin_of_node * P + slot_of_node  # device row, core-major

    # real edges only (self-loops handled separately)
    e_src = perm[row]
    e_dst = perm[col]
    e_nrm = (dinv[row] * dinv[col]).astype(np.float32)

    cd = e_dst // SEGR                      # dest core
    rd = e_dst % SEGR
    pd = rd // (2 * P)                      # pair in core
    sp = rd % (2 * P)                       # slot in pair (0..255)
    wn = e_src // WINR                      # source window
    wi = (e_src % WINR).astype(np.int64)    # index within window

    key = (cd * PAIRS + pd) * N_WIN + wn
    eo = np.argsort(key, kind="stable")
    key_s = key[eo]
    wi_s = wi[eo]
    sp_s = sp[eo]
    nr_s = e_nrm[eo]

    ncell = N_CORES * PAIRS * N_WIN
    cnt = np.bincount(key_s, minlength=ncell).reshape(N_CORES, PAIRS, N_WIN)
    starts = np.concatenate([[0], np.cumsum(cnt.ravel())[:-1]]).reshape(
        N_CORES, PAIRS, N_WIN)

    # uniform chunk counts per cell: max over cores
    cell_chunks = np.maximum(_ceil_div(cnt, P).max(axis=0), 1)  # [PAIRS, N_WIN]

    # program-level layout (shared by all cores)
    # per group g: [self chunks: 2 per pair][win 0 cells][win 1]...
    NSELF = 2
    call_len = np.zeros((N_GROUPS, N_WIN), np.int64)
    call_coff = np.zeros((N_GROUPS, N_WIN), np.int64)
    call_icol = np.zeros((N_GROUPS, N_WIN), np.int64)
    self_coff = np.zeros((N_GROUPS, GROUP_PAIRS), np.int64)
    pair_cell_chunk0 = np.zeros((PAIRS, N_WIN), np.int64)  # first chunk id
    grp_nch = np.zeros(N_GROUPS, np.int64)
    ic = 0
    for g in range(N_GROUPS):
        ch = 0  # chunk offset within group
        for pp in range(GROUP_PAIRS):
            self_coff[g, pp] = ch
            ch += NSELF
        for w in range(N_WIN):
            call_coff[g, w] = ch
            call_icol[g, w] = ic
            L = 0
            for pp in range(GROUP_PAIRS):
                p = g * GROUP_PAIRS + pp
                pair_cell_chunk0[p, w] = ch + L // P
                L += int(cell_chunks[p, w]) * P
            call_len[g, w] = L
            ch += L // P
            ic += L // 16
        grp_nch[g] = ch
    CPG = int(grp_nch.max())
    idx_cols = ic
    n_chunks_grp_total = int(grp_nch.sum())

    # per-core tables
    per_core = []
    for c in range(N_CORES):
        idx = np.zeros((16, idx_cols), np.int16)
        # dstf/nrm indexed by (group, chunk-in-group) flattened with CPG
        dstf = np.zeros((P, N_GROUPS * CPG), np.float32)
        nrmt = np.zeros((P, N_GROUPS * CPG), np.float32)
        for g in range(N_GROUPS):
            # self chunks: pair rows -> dest slots 0..255, self norm
            for pp in range(GROUP_PAIRS):
                p = g * GROUP_PAIRS + pp
                node0 = c * SEGR + p * 2 * P  # device row of pair start
                for k in range(NSELF):
                    chk = g * CPG + int(self_coff[g, pp]) + k
                    dstf[:, chk] = np.arange(k * P, (k + 1) * P,
                                             dtype=np.float32)
                    rows = node0 + k * P + np.arange(P)
                    nrmt[:, chk] = (dinv[:n_pad][...] if False else 0)
                    # self norm = dinv^2 of the node (1/deg); rows beyond
                    # n_nodes have no node -> 0
                    v = np.zeros(P, np.float32)
                    # invert perm lazily below
                    per = rows
                    nrmt[:, chk] = 0.0
                    dstf[:, chk] = np.arange(k * P, (k + 1) * P,
                                             dtype=np.float32)
            for w in range(N_WIN):
                L = int(call_len[g, w])
                lane0 = 0
                li = np.zeros(L, np.int64)
                ld = np.zeros(L, np.float64)
                ln = np.zeros(L, np.float64)
                for pp in range(GROUP_PAIRS):
                    p = g * GROUP_PAIRS + pp
                    n = int(cnt[c, p, w])
                    a = int(starts[c, p, w])
                    cap = int(cell_chunks[p, w]) * P
                    assert n <= cap, (c, p, w, n, cap)
                    li[lane0:lane0 + n] = wi_s[a:a + n]
                    ld[lane0:lane0 + n] = sp_s[a:a + n]
                    ln[lane0:lane0 + n] = nr_s[a:a + n]
                    lane0 += cap
                assert lane0 == L
                i0 = int(call_icol[g, w])
                idx[:, i0:i0 + L // 16] = li.reshape(L // 16, 16).T
                c0 = g * CPG + int(call_coff[g, w])
                for k in range(L // P):
                    dstf[:, c0 + k] = ld[k * P:(k + 1) * P]
                    nrmt[:, c0 + k] = ln[k * P:(k + 1) * P]
        per_core.append(dict(idx=idx, dstf=dstf, nrm=nrmt))

    # self-loop norms per node: 1/deg (dinv^2)
    self_nrm_dev = np.zeros(n_pad, np.float32)
    self_nrm_dev[perm[:n_nodes]] = (dinv * dinv).astype(np.float32)
    for c in range(N_CORES):
        dstf = per_core[c]["dstf"]
        nrmt = per_core[c]["nrm"]
        for g in range(N_GROUPS):
            for pp in range(GROUP_PAIRS):
                p = g * GROUP_PAIRS + pp
                node0 = c * SEGR + p * 2 * P
                for k in range(2):
                    chk = g * CPG + int(self_coff[g, pp]) + k
                    nrmt[:, chk] = self_nrm_dev[node0 + k * P:
                                                node0 + (k + 1) * P]

    # replicate idx rows to 128 partitions
    for c in range(N_CORES):
        per_core[c]["idx"] = np.tile(per_core[c]["idx"], (8, 1)).copy()

    prog = dict(call_len=call_len, call_coff=call_coff, call_icol=call_icol,
                self_coff=self_coff, pair_cell_chunk0=pair_cell_chunk0,
                cell_chunks=cell_chunks, CPG=CPG, idx_cols=idx_cols)
    return dict(perm=perm, n_pad=n_pad, cores=per_core, prog=prog)


def _pack_w(W, f_in_pad, f_out):
    import ml_dtypes
    f_in = W.shape[0]
    kc = _ceil_div(f_in_pad, P)
    Wp = np.zeros((kc * P, f_out), np.float32)
    Wp[:f_in] = np.asarray(W, np.float32)
    out = Wp.reshape(kc, P, f_out).transpose(1, 0, 2).reshape(P, kc * f_out)
    return out.astype(ml_dtypes.bfloat16).copy()


# --------------------------------------------------------------------------
# device program
# --------------------------------------------------------------------------

def _build_gcn(tc, ins, out_ap, cfg):
    nc = tc.nc
    prog = cfg["prog"]
    F1, H, O = F_IN_PAD, HIDDEN, F_OUT
    kc1 = F1 // P
    kc2 = H // P
    n_pad = cfg["n_pad"]
    core_id = None  # SPMD: own shard base differs per core — handled via
                    # the partition-id-free layout: own shard rows are the
                    # SAME local x2s tile; for layer 1 the own rows live in
                    # the input x at core-dependent offset — provided as a
                    # separate per-core input slice instead.
    call_len = prog["call_len"]
    call_coff = prog["call_coff"]
    call_icol = prog["call_icol"]
    self_coff = prog["self_coff"]
    pair_cell_chunk0 = prog["pair_cell_chunk0"]
    cell_chunks = prog["cell_chunks"]
    CPG = prog["CPG"]
    idx_cols = prog["idx_cols"]
    rg = [list(range(N_CORES))]

    ctx = ExitStack()
    with ctx:
        nc.gpsimd.load_library(library_config.mlp)

        const = ctx.enter_context(tc.tile_pool(name="const", bufs=1))
        dram = ctx.enter_context(tc.tile_pool(name="dram", bufs=1, space="DRAM"))
        work = ctx.enter_context(tc.tile_pool(name="work", bufs=3))
        psum = ctx.enter_context(tc.tile_pool(name="psum", bufs=2, space="PSUM"))

        def load_const(name, shape, dtype=F32):
            t = const.tile(list(shape), dtype, name=name)
            nc.sync.dma_start(out=t[:], in_=ins[name][:])
            return t

        iota_sb = load_const("iota", [P, 2 * P])
        ident_sb = load_const("ident", [P, P])
        ident_bf = const.tile([P, P], BF16, name="ident_bf")
        nc.vector.tensor_copy(out=ident_bf[:], in_=ident_sb[:])
        w1_sb = load_const("w1", [P, kc1 * H], BF16)
        w2_sb = load_const("w2", [P, kc2 * H], BF16)
        w3_sb = load_const("w3", [P, kc2 * O], BF16)
        b1_sb = load_const("b1bc", [P, H])
        b2_sb = load_const("b2bc", [P, H])
        b3_sb = load_const("b3row", [P, O])
        idx_sb = load_const("idx", [P, idx_cols], I16)
        dstf_sb = load_const("dstf", [P, N_GROUPS * CPG])
        nrm_sb = load_const("nrm", [P, N_GROUPS * CPG])

        outbuf = const.tile([P, TILES_PER_CORE * O], F32, name="outbuf")

        x2s = dram.tile([SEGR, H], BF16, name="x2s")
        x2f = dram.tile([n_pad, H], BF16, name="x2f", addr_space="Shared")
        zs = dram.tile([SEGR, ZPAD], BF16, name="zs")
        zf = dram.tile([n_pad, ZPAD], BF16, name="zf", addr_space="Shared")

        def gather_group(g, x_src_ap, own_ap, E, dt=BF16):
            """Staging tile for group g: self DMAs + 7 window gathers.
            E = row length in elements of dtype dt. The pool is bf16-typed
            and sized for 768B rows; other dtypes view it via bitcast."""
            stg_t = work.tile([P, CPG * H], BF16, name="stg", tag="stg", bufs=3)
            stg = stg_t[:] if dt == BF16 else stg_t[:].bitcast(dt)
            for pp in range(GROUP_PAIRS):
                p = g * GROUP_PAIRS + pp
                co = int(self_coff[g, pp])
                src = own_ap[p * 2 * P:(p + 1) * 2 * P, :E].rearrange(
                    "(c p) e -> p c e", p=P)
                dst = stg[:, co * E:(co + 2) * E].rearrange(
                    "p (c e) -> p c e", e=E)
                nc.sync.dma_start(out=dst, in_=src)
            for w in range(N_WIN):
                L = int(call_len[g, w])
                if L == 0:
                    continue
                co = int(call_coff[g, w])
                icol = int(call_icol[g, w])
                out3 = stg[:, co * E:(co + L // P) * E].rearrange(
                    "p (c e) -> p c e", e=E)
                nc.gpsimd.dma_gather(
                    out3,
                    x_src_ap[w * WINR:(w + 1) * WINR, :E],
                    idx_sb[:, icol:icol + L // 16],
                    L, L, E,
                )
            return stg

        def spmm_pair(g, pp, stg, E, kc):
            """aggT[f, 256 pair dests] (bf16) for pair p."""
            p = g * GROUP_PAIRS + pp
            chunks = [int(self_coff[g, pp]) + k for k in range(2)]
            for w in range(N_WIN):
                c0 = int(pair_cell_chunk0[p, w])
                for k in range(int(cell_chunks[p, w])):
                    chunks.append(c0 + k)
            mhs = []
            for chk in chunks:
                gch = g * CPG + chk
                mh = work.tile([P, 2 * P], BF16, name="mh", tag="mh", bufs=16)
                nc.vector.tensor_scalar(
                    out=mh[:],
                    in0=iota_sb[:],
                    scalar1=dstf_sb[:, gch:gch + 1],
                    scalar2=nrm_sb[:, gch:gch + 1],
                    op0=mybir.AluOpType.is_equal,
                    op1=mybir.AluOpType.mult,
                )
                mhs.append(mh)
            aggT = work.tile([P, kc2 * 2 * P], BF16, name="aggT", tag="aggT")
            for f in range(kc):
                pa = psum.tile([P, 2 * P], F32, name="pa", tag="pa", bufs=2)
                for i, chk in enumerate(chunks):
                    nc.tensor.matmul(
                        out=pa[:, :],
                        lhsT=stg[:, chk * E + f * P:chk * E + f * P + P],
                        rhs=mhs[i][:],
                        start=(i == 0),
                        stop=(i == len(chunks) - 1),
                    )
                nc.scalar.copy(out=aggT[:, f * 2 * P:(f + 1) * 2 * P],
                               in_=pa[:, :])
            return aggT

        def transform_tile(t, aggT, half, kc_in, w_sb, b_sb, x_next_shard,
                           zexp=None, tig=None):
            pt = psum.tile([P, H], F32, name="pt", tag="pt", bufs=2)
            for k in range(kc_in):
                nc.tensor.matmul(
                    out=pt[:, :H],
                    lhsT=aggT[:, k * 2 * P + half * P:k * 2 * P + half * P + P],
                    rhs=w_sb[:, k * H:(k + 1) * H],
                    start=(k == 0),
                    stop=(k == kc_in - 1),
                )
            nc.vector.tensor_tensor(
                out=pt[:, :H], in0=pt[:, :H], in1=b_sb[:],
                op=mybir.AluOpType.add,
            )
            xrow = work.tile([P, H], BF16, name="xrow", tag="xrow")
            nc.scalar.activation(
                out=xrow[:], in_=pt[:, :H],
                func=mybir.ActivationFunctionType.Relu,
            )
            if x_next_shard is not None:
                nc.sync.dma_start(out=x_next_shard[t * P:(t + 1) * P, :],
                                  in_=xrow[:])
            if zexp is not None:
                # z_tile[dest, 2] = xrow @ W3 via 3 PE transposes
                xT = work.tile([P, kc2 * P], BF16, name="xT", tag="xT")
                for j in range(kc2):
                    ptp = psum.tile([P, P], BF16, name="ptp", tag="ptp",
                                    bufs=2)
                    nc.tensor.transpose(
                        out=ptp[:],
                        in_=xrow[:, j * P:(j + 1) * P],
                        identity=ident_bf[:],
                    )
                    nc.scalar.copy(out=xT[:, j * P:(j + 1) * P], in_=ptp[:])
                zp = psum.tile([P, O], F32, name="zp", tag="zp", bufs=2)
                for j in range(kc2):
                    nc.tensor.matmul(
                        out=zp[:, :O],
                        lhsT=xT[:, j * P:(j + 1) * P],
                        rhs=w3_sb[:, j * O:(j + 1) * O],
                        start=(j == 0),
                        stop=(j == kc2 - 1),
                    )
                nc.vector.tensor_copy(out=zexp[:, tig * ZPAD:tig * ZPAD + O],
                                      in_=zp[:, :O])

        def layer(x_src_ap, own_ap, E, kc, w_sb, b_sb, x_next_shard,
                  x_next_full, z_out=None):
            for g in range(N_GROUPS):
                stg = gather_group(g, x_src_ap, own_ap, E)
                zexp = None
                if z_out is not None:
                    zexp = work.tile([P, 2 * GROUP_PAIRS * ZPAD], BF16,
                                     name="zexp", tag="zexp", bufs=2)
                    nc.vector.memset(zexp[:], 0.0)
                for pp in range(GROUP_PAIRS):
                    p = g * GROUP_PAIRS + pp
                    aggT = spmm_pair(g, pp, stg, E, kc)
                    transform_tile(2 * p, aggT, 0, kc, w_sb, b_sb,
                                   x_next_shard, zexp, 2 * pp)
                    transform_tile(2 * p + 1, aggT, 1, kc, w_sb, b_sb,
                                   x_next_shard, zexp, 2 * pp + 1)
                if z_out is not None:
                    nt = 2 * GROUP_PAIRS
                    dst = z_out[g * nt * P:(g + 1) * nt * P, :].rearrange(
                        "(t p) e -> p t e", p=P)
                    src = zexp[:].rearrange("p (t e) -> p t e", e=ZPAD)
                    nc.sync.dma_start(out=dst, in_=src)
            if x_next_full is not None and not NO_AG:
                nc.gpsimd.collective_compute(
                    "AllGather", mybir.AluOpType.bypass, replica_groups=rg,
                    ins=[x_next_shard.opt()], outs=[x_next_full.opt()],
                )

        layer(ins["x"][:], ins["xown"][:], F1, kc1, w1_sb, b1_sb, x2s, x2f)
        # layer 2 also produces z = relu(.)W3 per own tile, AllGathered small
        layer(x2f[:], x2s[:], H, kc2, w2_sb, b2_sb, None, None, z_out=zs)
        if not NO_AG:
            nc.gpsimd.collective_compute(
                "AllGather", mybir.AluOpType.bypass, replica_groups=rg,
                ins=[zs.opt()], outs=[zf.opt()],
            )

        # layer 3: aggregate the 2-col projections directly
        for g in range(N_GROUPS):
            stg = gather_group(g, zf[:], zs[:], ZPAD, dt=BF16)
            for pp in range(GROUP_PAIRS):
                p = g * GROUP_PAIRS + pp
                chunks = [int(self_coff[g, pp]) + k for k in range(2)]
                for w in range(N_WIN):
                    c0 = int(pair_cell_chunk0[p, w])
                    for k in range(int(cell_chunks[p, w])):
                        chunks.append(c0 + k)
                mhs = []
                for chk in chunks:
                    gch = g * CPG + chk
                    mh = work.tile([P, 2 * P], BF16, name="mh", tag="mh",
                                   bufs=16)
                    nc.vector.tensor_scalar(
                        out=mh[:],
                        in0=iota_sb[:],
                        scalar1=dstf_sb[:, gch:gch + 1],
                        scalar2=nrm_sb[:, gch:gch + 1],
                        op0=mybir.AluOpType.is_equal,
                        op1=mybir.AluOpType.mult,
                    )
                    mhs.append(mh)
                for half in range(2):
                    t = 2 * p + half
                    po = psum.tile([P, O], F32, name="po", tag="zp", bufs=2)
                    for i, chk in enumerate(chunks):
                        nc.tensor.matmul(
                            out=po[:, :O],
                            lhsT=mhs[i][:, half * P:half * P + P],
                            rhs=stg[:, chk * ZPAD:chk * ZPAD + O],
                            start=(i == 0),
                            stop=(i == len(chunks) - 1),
                        )
                    nc.vector.tensor_tensor(
                        out=outbuf[:, t * O:(t + 1) * O],
                        in0=po[:, :O],
                        in1=b3_sb[:],
                        op=mybir.AluOpType.add,
                    )
        nc.sync.dma_start(out=out_ap, in_=outbuf[:])


# --------------------------------------------------------------------------
# execution (axon / PJRT path with pipelined timing)
# --------------------------------------------------------------------------

def _run_pjrt_timed(nc, in_maps, n_cores, time_iters=0):
    global EXEC_NS
    import jax
    from jax.experimental.shard_map import shard_map
    from jax.sharding import Mesh, NamedSharding, PartitionSpec

    from concourse import bass2jax as b2j

    b2j.install_neuronx_cc_hook()

    partition_name = (nc.partition_id_tensor.name
                      if nc.partition_id_tensor else None)
    in_names, out_names, out_avals, zero_outs = [], [], [], []
    for alloc in nc.m.functions[0].allocations:
        if not isinstance(alloc, mybir.MemoryLocationSet):
            continue
        name = alloc.memorylocations[0].name
        if alloc.kind == "ExternalInput":
            if name != partition_name:
                in_names.append(name)
        elif alloc.kind == "ExternalOutput":
            out_names.append(name)
            shape = tuple(alloc.tensor_shape)
            dtype = mybir.dt.np(alloc.dtype)
            out_avals.append(jax.core.ShapedArray(shape, dtype))
            zero_outs.append(np.zeros(shape, dtype))
    n_params = len(in_names)
    all_in_names = list(in_names) + list(out_names)
    if partition_name is not None:
        all_in_names.append(partition_name)
    all_in_names = tuple(all_in_names)

    def _body(*args):
        operands = list(args)
        if partition_name is not None:
            operands.append(b2j.partition_id_tensor())
        outs = b2j._bass_exec_p.bind(
            *operands,
            out_avals=tuple(out_avals),
            in_names=all_in_names,
            out_names=tuple(out_names),
            lowering_input_output_aliases=(),
            sim_require_finite=True,
            sim_require_nnan=True,
            nc=nc,
        )
        return tuple(outs)

    devices = jax.devices()[:n_cores]
    assert len(devices) == n_cores
    mesh = Mesh(np.asarray(devices), ("core",))
    spec = PartitionSpec("core")
    n_all = n_params + len(zero_outs)
    jitted = jax.jit(shard_map(
        _body, mesh=mesh, in_specs=(spec,) * n_all,
        out_specs=(spec,) * len(out_names), check_rep=False))

    sharding = NamedSharding(mesh, spec)
    g_in = [
        jax.device_put(
            np.concatenate([np.asarray(in_maps[c][nm]) for c in range(n_cores)],
                           axis=0), sharding)
        for nm in in_names
    ]
    g_zero = [
        jax.device_put(np.concatenate([z] * n_cores, axis=0), sharding)
        for z in zero_outs
    ]

    out_arrs = jitted(*g_in, *g_zero)
    jax.block_until_ready(out_arrs)
    results = [
        {nm: np.asarray(out_arrs[i]).reshape(n_cores, *out_avals[i].shape)[c]
         for i, nm in enumerate(out_names)}
        for c in range(n_cores)
    ]

    if time_iters > 0:
        def timed(n_iter):
            t0 = time.perf_counter()
            o = None
            for _ in range(n_iter):
                o = jitted(*g_in, *g_zero)
            jax.block_until_ready(o)
            return time.perf_counter() - t0

        timed(2)
        best = None
        for _ in range(max(3, time_iters // 3)):
            w4 = timed(4)
            w16 = timed(16)
            per_iter = (w16 - w4) / 12
            if best is None or per_iter < best:
                best = per_iter
            print(f"[timing] w4={w4*1e3:.2f}ms w16={w16*1e3:.2f}ms "
                  f"per-iter={per_iter*1e3:.3f}ms")
        EXEC_NS = int(best * 1e9)
    return results


# --------------------------------------------------------------------------
# top level
# --------------------------------------------------------------------------

def build_and_inputs(x, edge_index, W1, b1, W2, b2, W3, b3,
                     tiles_per_core=None):
    """Build the Bass program + per-core input maps (shared by kernel()
    and the small-sim test)."""
    import ml_dtypes
    x = np.asarray(x, np.float32)
    edge_index = np.asarray(edge_index)
    n_nodes = x.shape[0]

    pre = _preprocess(edge_index, n_nodes)
    n_pad = pre["n_pad"]
    prog = pre["prog"]

    x_dev = np.zeros((n_pad, F_IN_PAD), ml_dtypes.bfloat16)
    x_dev[pre["perm"][:n_nodes], :F_IN] = x

    common = dict(
        x=x_dev,
        iota=np.tile(np.arange(2 * P, dtype=np.float32), (P, 1)).copy(),
        ident=np.eye(P, dtype=np.float32),
        w1=_pack_w(W1, F_IN_PAD, HIDDEN),
        w2=_pack_w(W2, HIDDEN, HIDDEN),
        w3=_pack_w(W3, HIDDEN, F_OUT),
        b1bc=np.tile(np.asarray(b1, np.float32), (P, 1)).copy(),
        b2bc=np.tile(np.asarray(b2, np.float32), (P, 1)).copy(),
        b3row=np.tile(np.asarray(b3, np.float32), (P, 1)).copy(),
    )
    in_maps = []
    for c in range(N_CORES):
        m = dict(common)
        cm = pre["cores"][c]
        m["idx"] = cm["idx"]
        m["dstf"] = cm["dstf"]
        m["nrm"] = cm["nrm"]
        m["xown"] = x_dev[c * SEGR:(c + 1) * SEGR].copy()
        in_maps.append(m)

    nc = bacc.Bacc("TRN2", target_bir_lowering=False, debug=False,
                   enable_asserts=False, num_devices=N_CORES)
    ins_aps = {}
    for name, arr in in_maps[0].items():
        ins_aps[name] = nc.dram_tensor(
            name, list(arr.shape), mybir.dt.from_np(arr.dtype),
            kind="ExternalInput").ap()
    out_t = nc.dram_tensor("out", [P, TILES_PER_CORE * F_OUT], F32,
                           kind="ExternalOutput")

    cfg = dict(n_pad=n_pad, prog=prog)
    with tile.TileContext(nc) as tc:
        _build_gcn(tc, ins_aps, out_t.ap(), cfg)
    nc.compile()
    return nc, in_maps, pre


def kernel(x, edge_index, W1, b1, W2, b2, W3, b3, _trace=False, _time_iters=0):
    global LAST_RESULTS
    n_nodes = np.asarray(x).shape[0]
    assert n_nodes == N_NODES

    nc, in_maps, pre = build_and_inputs(x, edge_index, W1, b1, W2, b2, W3, b3)
    n_pad = pre["n_pad"]

    if axon_active():
        results = _run_pjrt_timed(nc, in_maps, N_CORES, time_iters=_time_iters)
    else:
        res = bass_utils.run_bass_kernel_spmd(
            nc, in_maps, core_ids=list(range(N_CORES)), trace=_trace)
        LAST_RESULTS = res
        results = res.results

    out_dev = np.zeros((n_pad, F_OUT), np.float32)
    T = TILES_PER_CORE
    for c in range(N_CORES):
        o = results[c]["out"]
        rows = o.reshape(P, T, F_OUT).transpose(1, 0, 2).reshape(T * P, F_OUT)
        out_dev[c * T * P:(c + 1) * T * P] = rows
    return out_dev[pre["perm"][:n_nodes]].copy()
